# revision 58
# baseline (speedup 1.0000x reference)
"""Trainium2 Bass kernel for nn_AttentionFFM (Vector-engine-bound; ~190-245 us
vs 295-351 us baseline on the same HW epochs).

Reference, per token (b, k) with v = x[b, :, k] (64-vector) and constant
symmetric w = vk @ vk.T:

    e_ij  = exp(v_i * v_j * w_ij)
    out_i = v_i * (sum_j v_j e_ij) / (sum_j e_ij)

The v_i factor is pulled OUT of the softmax-weighted sum, so the naive
kernel's full-size s = v_i*v_j outer product (fp32 strided, 8.4 us/instr)
and q = s*e pass disappear; only j-varying factors touch 64x64 tensors.

Layout (per core; batch-parallel across 8 cores, BL=128 batches each):
partitions = batch b, free = (k2, i, j) for one k-PAIR at a time (k-pair
packing halves instruction count). All big ops are bf16 with stride-1
innermost APs => DVE 2x mode (0.52 ns/elem; no 4x mode exists for
tensor_tensor on this HW). Per pair, with e2 = [src=2, k2, i, j] tile:

    vi_rep   = v_i bcast along j     [ACT Copy: ACT cost is stride-
                                      independent, so the stride-0
                                      broadcast that would force 1x on
                                      DVE rides the half-idle ACT engine]
    m1       = w (*) v_j-bcast       [DVE 2x, broadcast AP is packed]
    e2[:,0]  = m1 (*) vi_rep  (= z)  [DVE 2x]
    e2[:,1]  = exp(e2[:,0])   (= e)  [ACT]
    e2[:,0]  = e2[:,1] (*) v_j (= t) [DVE 2x, overwrites z]
    one merged pairwise-halving bf16 add-tree over e2 rows (s,k2,i)=256
    reduces BOTH tensors' j-dims in 5 double-wide 2x levels, then two
    small adds split the results into n_all / d_all columns.
    (Measured: tensor_reduce is 1x, tensor_tensor_scan 0.5x, GpSimd adds
    4-7 ns/elem, bn_stats caps at 512 free — trees win.)

Final, batched over all 16 k in fp32:
    out = v * N * reciprocal_approx_fast(D)    (5x faster than
                                                vector.reciprocal)

Host-side prep: x is passed k-major ((k,i) columns, bf16) so v-vectors are
contiguous per k; output is returned k-major and transposed back on host.
All tiles are allocated ONCE (tile-pool slot cycling faulted on this HW in
a previous session).
"""

import sys
from contextlib import ExitStack

import numpy as np

if "/opt/trn_rl_repo" not in sys.path:
    sys.path.insert(0, "/opt/trn_rl_repo")

import concourse.bass as bass
import concourse.tile as tile
from concourse import bacc, mybir
from concourse.bass_utils import run_bass_kernel_spmd

_NEFF_CACHE_DIR = "/tmp/bass_neff_cache"


def _install_neff_cache():
    import hashlib
    import shutil

    from concourse import bass_utils as _bu

    if getattr(_bu.compile_bir_kernel, "_is_cached_wrapper", False):
        return

    _orig = _bu.compile_bir_kernel

    _volatile = {"ant_traceback", "filename", "lineno", "kernel_name"}

    def _strip(obj):
        if isinstance(obj, dict):
            return {k: _strip(v) for k, v in obj.items() if k not in _volatile}
        if isinstance(obj, list):
            return [_strip(v) for v in obj]
        return obj

    def _key(bir_json):
        import orjson

        try:
            normalized = orjson.dumps(_strip(orjson.loads(bir_json)))
        except Exception:
            normalized = bir_json
        return hashlib.sha256(normalized).hexdigest()[:32]

    def _cached(bir_json, tmpdir, neff_name="file.neff"):
        import os as _os

        try:
            _os.makedirs(_NEFF_CACHE_DIR, exist_ok=True)
            p = _os.path.join(_NEFF_CACHE_DIR, _key(bir_json) + ".neff")
            dst = _os.path.join(tmpdir, neff_name)
            if _os.path.exists(p):
                shutil.copy(p, dst)
                return dst
            out = _orig(bir_json, tmpdir, neff_name)
            try:
                shutil.copy(out, p)
            except Exception:
                pass
            return out
        except Exception:
            return _orig(bir_json, tmpdir, neff_name)

    _cached._is_cached_wrapper = True
    _bu.compile_bir_kernel = _cached
    try:
        import concourse.bass2jax as _b2j

        if hasattr(_b2j, "compile_bir_kernel"):
            _b2j.compile_bir_kernel = _cached
    except Exception:
        pass


_install_neff_cache()

B, M, K = 1024, 64, 16
NCORES = 8
BL = B // NCORES
NPAIR = K // 2

_CACHE = {}
LAST_RESULTS = None
TRACE = False
# "act": materialize the v_i broadcast with an ACT-engine Copy (stride-
#        independent cost, idle capacity) so the z multiply runs 2x on DVE.
# "vector": read the broadcast AP directly in the z multiply (1x, fallback).
VI_ENGINE = "act"


def _build():
    nc = bacc.Bacc(
        "TRN2",
        target_bir_lowering=False,
        debug=False,
        num_devices=NCORES,
    )
    # x, k-major bf16: xb[b, k*M + i] = x[b, i, k]
    x_in = nc.declare_dram_parameter(
        "x", [BL, K * M], mybir.dt.bfloat16, isOutput=False
    )
    # w arrives pre-broadcast across partitions so the load is a plain
    # contiguous hardware-DGE DMA (the gpsimd software-ring partition
    # broadcast serialized ~4 us into the startup critical path).
    w_in = nc.declare_dram_parameter(
        "w", [BL, M * M], mybir.dt.bfloat16, isOutput=False
    )
    # out, k-major fp32: out[b, k*M + i]
    out_ext = nc.declare_dram_parameter(
        "out", [BL, K * M], mybir.dt.float32, isOutput=True
    )

    with tile.TileContext(nc) as tc, ExitStack() as ctx:
        const = ctx.enter_context(tc.tile_pool(name="const", bufs=1))
        big = ctx.enter_context(tc.tile_pool(name="big", bufs=1))

        xb_sb = const.tile([BL, K * M], mybir.dt.bfloat16)
        nc.sync.dma_start(out=xb_sb[:, :], in_=x_in[:, :])

        w_sb = const.tile([BL, M * M], mybir.dt.bfloat16)
        nc.scalar.dma_start(out=w_sb[:, :], in_=w_in[:, :])

        out_sb = const.tile([BL, K * M], mybir.dt.float32)

        xb_3d = xb_sb[:, :].rearrange("p (k i) -> p k i", i=M)
        w4 = (
            w_sb[:, :]
            .rearrange("p (i j) -> p i j", j=M)
            .unsqueeze(1)
            .broadcast_to((BL, 2, M, M))
        )

        # Ping-pong tiles where producer/consumer engines differ; the m1
        # tile is produced and consumed back-to-back on Vector, so a
        # single buffer costs no overlap.
        vi_t = (
            [
                big.tile(
                    [BL, 2, M, M], mybir.dt.bfloat16, tag=f"vi{p}", name=f"vi{p}"
                )
                for p in range(2)
            ]
            if VI_ENGINE == "act"
            else None
        )
        m1 = big.tile([BL, 2, M, M], mybir.dt.bfloat16, tag="m1", name="m1")
        e2_t = [
            big.tile(
                [BL, 2, 2, M, M], mybir.dt.bfloat16, tag=f"e2{p}", name=f"e2{p}"
            )
            for p in range(2)
        ]
        # One shared merged-tree tile set (all tree levels run on Vector,
        # which serializes them anyway).
        tr2 = {}
        width = M // 2
        while width >= 2:
            tr2[width] = big.tile(
                [BL, 4 * M, width],
                mybir.dt.bfloat16,
                tag=f"tr{width}",
                name=f"tr{width}",
            )
            width //= 2

        d_all = const.tile([BL, K * M], mybir.dt.float32, tag="dall", name="dall")
        n_all = const.tile([BL, K * M], mybir.dt.bfloat16, tag="nall", name="nall")
        rd_all = const.tile([BL, K * M], mybir.dt.float32, tag="rdall", name="rdall")
        p_all = const.tile([BL, K * M], mybir.dt.float32, tag="pall", name="pall")

        for g in range(NPAIR):
            par = g % 2
            xp = xb_3d[:, 2 * g : 2 * g + 2, :]  # [BL, 2, M] contiguous
            vj = xp.unsqueeze(-2).broadcast_to((BL, 2, M, M))  # j innermost, s=1
            vi = xp.unsqueeze(-1).broadcast_to((BL, 2, M, M))  # j innermost, s=0

            e2 = e2_t[par]
            z_sl = e2[:, 0, :, :, :]  # holds z, then t
            e_sl = e2[:, 1, :, :, :]  # holds e

            nc.vector.tensor_tensor(
                out=m1[:, :, :, :], in0=w4, in1=vj, op=mybir.AluOpType.mult
            )

            if VI_ENGINE == "act":
                vi_rep = vi_t[par]
                nc.scalar.copy(out=vi_rep[:, :, :, :], in_=vi)
                vi_in = vi_rep[:, :, :, :]
            else:
                vi_in = vi  # direct broadcast AP (1x mode)
            nc.vector.tensor_tensor(
                out=z_sl, in0=m1[:, :, :, :], in1=vi_in, op=mybir.AluOpType.mult
            )

            nc.scalar.activation(
                out=e_sl.rearrange("p a i j -> p (a i j)"),
                in_=z_sl.rearrange("p a i j -> p (a i j)"),
                func=mybir.ActivationFunctionType.Exp,
            )

            if g == NPAIR - 1:
                # Early final chain for pairs 0..6: queued here so it
                # executes in the Vector gap while ACT runs the last
                # pair's exp (measured ~3.9 us idle otherwise).
                ec = slice(0, (NPAIR - 1) * 2 * M)
                nc.vector.reciprocal_approx_fast(
                    out=rd_all[:, ec], in_=d_all[:, ec]
                )
                nc.vector.tensor_tensor(
                    out=p_all[:, ec],
                    in0=n_all[:, ec],
                    in1=rd_all[:, ec],
                    op=mybir.AluOpType.mult,
                )
                nc.vector.tensor_tensor(
                    out=out_sb[:, ec],
                    in0=p_all[:, ec],
                    in1=xb_sb[:, ec],
                    op=mybir.AluOpType.mult,
                )

            # t = e * v_j overwrites the z slice; rows (s=0) become the
            # numerator stream, rows (s=1) the denominator stream.
            nc.vector.tensor_tensor(
                out=z_sl, in0=e_sl, in1=vj, op=mybir.AluOpType.mult
            )

            # One merged tree over 256 rows = (src 2, k2 2, i 64).
            cur = e2[:, :, :, :, :].rearrange("p s a i j -> p (s a i) j")
            width = M // 2
            while width >= 2:
                nxt = tr2[width][:, :, :]
                nc.vector.tensor_tensor(
                    out=nxt,
                    in0=cur[:, :, 0:width],
                    in1=cur[:, :, width : 2 * width],
                    op=mybir.AluOpType.add,
                )
                cur = nxt
                width //= 2
            cols = slice(g * 2 * M, (g + 1) * 2 * M)
            nc.vector.tensor_tensor(
                out=d_all[:, cols],
                in0=cur[:, 2 * M : 4 * M, 0],
                in1=cur[:, 2 * M : 4 * M, 1],
                op=mybir.AluOpType.add,
            )
            nc.vector.tensor_tensor(
                out=n_all[:, cols],
                in0=cur[:, 0 : 2 * M, 0],
                in1=cur[:, 0 : 2 * M, 1],
                op=mybir.AluOpType.add,
            )

        # Late final chain: only the last pair's 128 columns remain.
        lc = slice((NPAIR - 1) * 2 * M, K * M)
        nc.vector.reciprocal_approx_fast(out=rd_all[:, lc], in_=d_all[:, lc])
        nc.vector.tensor_tensor(
            out=p_all[:, lc],
            in0=n_all[:, lc],
            in1=rd_all[:, lc],
            op=mybir.AluOpType.mult,
        )
        nc.vector.tensor_tensor(
            out=out_sb[:, lc],
            in0=p_all[:, lc],
            in1=xb_sb[:, lc],
            op=mybir.AluOpType.mult,
        )

        nc.sync.dma_start(out=out_ext[:, :], in_=out_sb[:, :])

    nc.compile()
    return nc


def _get_nc():
    key = ("nc", VI_ENGINE)
    if key not in _CACHE:
        _CACHE[key] = _build()
    return _CACHE[key]


def kernel(x, vk):
    global LAST_RESULTS
    x = np.ascontiguousarray(np.asarray(x), dtype=np.float32)
    vk = np.ascontiguousarray(np.asarray(vk), dtype=np.float32)
    assert x.shape == (B, M, K) and vk.shape[0] == M

    import ml_dtypes

    bf16 = ml_dtypes.bfloat16
    w = np.ascontiguousarray(
        np.broadcast_to((vk @ vk.T).astype(bf16).reshape(1, M * M), (BL, M * M))
    )
    # k-major bf16 per core: xb[b, k*M + i] = x[b, i, k]
    xs = x.reshape(NCORES, BL, M, K)
    in_maps = []
    for i in range(NCORES):
        xb = np.ascontiguousarray(xs[i].transpose(0, 2, 1)).reshape(BL, K * M)
        in_maps.append({"x": xb.astype(bf16), "w": w})

    nc = _get_nc()
    res = run_bass_kernel_spmd(nc, in_maps, core_ids=list(range(NCORES)), trace=TRACE)
    LAST_RESULTS = res
    outs = []
    for i in range(NCORES):
        o = np.asarray(res.results[i]["out"]).reshape(BL, K, M)
        outs.append(o.transpose(0, 2, 1))  # -> [BL, M, K]
    out = np.concatenate(outs, axis=0)
    return np.ascontiguousarray(out).astype(np.float32, copy=False)


# revision 68
# speedup vs baseline: 1.0007x; 1.0007x over previous
"""Trainium2 Bass kernel for nn_AttentionFFM (Vector-engine-bound; ~190-245 us
vs 295-351 us baseline on the same HW epochs).

Reference, per token (b, k) with v = x[b, :, k] (64-vector) and constant
symmetric w = vk @ vk.T:

    e_ij  = exp(v_i * v_j * w_ij)
    out_i = v_i * (sum_j v_j e_ij) / (sum_j e_ij)

The v_i factor is pulled OUT of the softmax-weighted sum, so the naive
kernel's full-size s = v_i*v_j outer product (fp32 strided, 8.4 us/instr)
and q = s*e pass disappear; only j-varying factors touch 64x64 tensors.

Layout (per core; batch-parallel across 8 cores, BL=128 batches each):
partitions = batch b, free = (k2, i, j) for one k-PAIR at a time (k-pair
packing halves instruction count). All big ops are bf16 with stride-1
innermost APs => DVE 2x mode (0.52 ns/elem; no 4x mode exists for
tensor_tensor on this HW). Per pair, with e2 = [src=2, k2, i, j] tile:

    vi_rep   = v_i bcast along j     [ACT Copy: ACT cost is stride-
                                      independent, so the stride-0
                                      broadcast that would force 1x on
                                      DVE rides the half-idle ACT engine]
    m1       = w (*) v_j-bcast       [DVE 2x, broadcast AP is packed]
    e2[:,0]  = m1 (*) vi_rep  (= z)  [DVE 2x]
    e2[:,1]  = exp(e2[:,0])   (= e)  [ACT]
    e2[:,0]  = e2[:,1] (*) v_j (= t) [DVE 2x, overwrites z]
    one merged pairwise-halving bf16 add-tree over e2 rows (s,k2,i)=256
    reduces BOTH tensors' j-dims in 5 double-wide 2x levels, then two
    small adds split the results into n_all / d_all columns.
    (Measured: tensor_reduce is 1x, tensor_tensor_scan 0.5x, GpSimd adds
    4-7 ns/elem, bn_stats caps at 512 free — trees win.)

Final, batched over all 16 k in fp32:
    out = v * N * reciprocal_approx_fast(D)    (5x faster than
                                                vector.reciprocal)

Host-side prep: x is passed k-major ((k,i) columns, bf16) so v-vectors are
contiguous per k; output is returned k-major and transposed back on host.
All tiles are allocated ONCE (tile-pool slot cycling faulted on this HW in
a previous session).
"""

import sys
from contextlib import ExitStack

import numpy as np

if "/opt/trn_rl_repo" not in sys.path:
    sys.path.insert(0, "/opt/trn_rl_repo")

import concourse.bass as bass
import concourse.tile as tile
from concourse import bacc, mybir
from concourse.bass_utils import run_bass_kernel_spmd

_NEFF_CACHE_DIR = "/tmp/bass_neff_cache"


def _install_neff_cache():
    import hashlib
    import shutil

    from concourse import bass_utils as _bu

    if getattr(_bu.compile_bir_kernel, "_is_cached_wrapper", False):
        return

    _orig = _bu.compile_bir_kernel

    _volatile = {"ant_traceback", "filename", "lineno", "kernel_name"}

    def _strip(obj):
        if isinstance(obj, dict):
            return {k: _strip(v) for k, v in obj.items() if k not in _volatile}
        if isinstance(obj, list):
            return [_strip(v) for v in obj]
        return obj

    def _key(bir_json):
        import orjson

        try:
            normalized = orjson.dumps(_strip(orjson.loads(bir_json)))
        except Exception:
            normalized = bir_json
        return hashlib.sha256(normalized).hexdigest()[:32]

    def _cached(bir_json, tmpdir, neff_name="file.neff"):
        import os as _os

        try:
            _os.makedirs(_NEFF_CACHE_DIR, exist_ok=True)
            p = _os.path.join(_NEFF_CACHE_DIR, _key(bir_json) + ".neff")
            dst = _os.path.join(tmpdir, neff_name)
            if _os.path.exists(p):
                shutil.copy(p, dst)
                return dst
            out = _orig(bir_json, tmpdir, neff_name)
            try:
                shutil.copy(out, p)
            except Exception:
                pass
            return out
        except Exception:
            return _orig(bir_json, tmpdir, neff_name)

    _cached._is_cached_wrapper = True
    _bu.compile_bir_kernel = _cached
    try:
        import concourse.bass2jax as _b2j

        if hasattr(_b2j, "compile_bir_kernel"):
            _b2j.compile_bir_kernel = _cached
    except Exception:
        pass


_install_neff_cache()

B, M, K = 1024, 64, 16
NCORES = 8
BL = B // NCORES
NPAIR = K // 2

_CACHE = {}
LAST_RESULTS = None
TRACE = False
# "act": materialize the v_i broadcast with an ACT-engine Copy (stride-
#        independent cost, idle capacity) so the z multiply runs 2x on DVE.
# "vector": read the broadcast AP directly in the z multiply (1x, fallback).
VI_ENGINE = "act"


def _build():
    nc = bacc.Bacc(
        "TRN2",
        target_bir_lowering=False,
        debug=False,
        num_devices=NCORES,
    )
    # x, k-major bf16: xb[b, k*M + i] = x[b, i, k]
    x_in = nc.declare_dram_parameter(
        "x", [BL, K * M], mybir.dt.bfloat16, isOutput=False
    )
    # w arrives pre-broadcast across partitions so the load is a plain
    # contiguous hardware-DGE DMA (the gpsimd software-ring partition
    # broadcast serialized ~4 us into the startup critical path).
    w_in = nc.declare_dram_parameter(
        "w", [BL, M * M], mybir.dt.bfloat16, isOutput=False
    )
    # out, k-major bf16 (host upcasts to fp32 exactly; halves output DMA)
    out_ext = nc.declare_dram_parameter(
        "out", [BL, K * M], mybir.dt.bfloat16, isOutput=True
    )

    with tile.TileContext(nc) as tc, ExitStack() as ctx:
        const = ctx.enter_context(tc.tile_pool(name="const", bufs=1))
        big = ctx.enter_context(tc.tile_pool(name="big", bufs=1))

        xb_sb = const.tile([BL, K * M], mybir.dt.bfloat16)
        nc.sync.dma_start(out=xb_sb[:, :], in_=x_in[:, :])

        w_sb = const.tile([BL, M * M], mybir.dt.bfloat16)
        nc.scalar.dma_start(out=w_sb[:, :], in_=w_in[:, :])

        out_sb = const.tile([BL, K * M], mybir.dt.bfloat16)

        xb_3d = xb_sb[:, :].rearrange("p (k i) -> p k i", i=M)
        w4 = (
            w_sb[:, :]
            .rearrange("p (i j) -> p i j", j=M)
            .unsqueeze(1)
            .broadcast_to((BL, 2, M, M))
        )

        # Ping-pong tiles where producer/consumer engines differ; the m1
        # tile is produced and consumed back-to-back on Vector, so a
        # single buffer costs no overlap.
        vi_t = (
            [
                big.tile(
                    [BL, 2, M, M], mybir.dt.bfloat16, tag=f"vi{p}", name=f"vi{p}"
                )
                for p in range(2)
            ]
            if VI_ENGINE == "act"
            else None
        )
        m1 = big.tile([BL, 4, M, M], mybir.dt.bfloat16, tag="m1", name="m1")
        e2_t = [
            big.tile(
                [BL, 2, 2, M, M], mybir.dt.bfloat16, tag=f"e2{p}", name=f"e2{p}"
            )
            for p in range(2)
        ]
        # One shared merged-tree tile set (all tree levels run on Vector,
        # which serializes them anyway).
        tr2 = {}
        width = M // 2
        while width >= 2:
            tr2[width] = big.tile(
                [BL, 4 * M, width],
                mybir.dt.bfloat16,
                tag=f"tr{width}",
                name=f"tr{width}",
            )
            width //= 2

        d_all = const.tile([BL, K * M], mybir.dt.float32, tag="dall", name="dall")
        n_all = const.tile([BL, K * M], mybir.dt.bfloat16, tag="nall", name="nall")
        rd_all = const.tile([BL, K * M], mybir.dt.float32, tag="rdall", name="rdall")
        rd_bf = const.tile([BL, K * M], mybir.dt.bfloat16, tag="rdbf", name="rdbf")
        p_bf = const.tile([BL, K * M], mybir.dt.bfloat16, tag="pbf", name="pbf")

        def final_chain(sl):
            """N/D ratio and v-multiply for column slice sl; the fp32
            reciprocal is downcast on ACT so both multiplies run 2x."""
            nc.vector.reciprocal_approx_fast(out=rd_all[:, sl], in_=d_all[:, sl])
            nc.scalar.copy(out=rd_bf[:, sl], in_=rd_all[:, sl])
            nc.vector.tensor_tensor(
                out=p_bf[:, sl],
                in0=n_all[:, sl],
                in1=rd_bf[:, sl],
                op=mybir.AluOpType.mult,
            )
            nc.vector.tensor_tensor(
                out=out_sb[:, sl],
                in0=p_bf[:, sl],
                in1=xb_sb[:, sl],
                op=mybir.AluOpType.mult,
            )

        for g in range(NPAIR):
            par = g % 2
            xp = xb_3d[:, 2 * g : 2 * g + 2, :]  # [BL, 2, M] contiguous
            vj = xp.unsqueeze(-2).broadcast_to((BL, 2, M, M))  # j innermost, s=1
            vi = xp.unsqueeze(-1).broadcast_to((BL, 2, M, M))  # j innermost, s=0

            e2 = e2_t[par]
            z_sl = e2[:, 0, :, :, :]  # holds z, then t
            e_sl = e2[:, 1, :, :, :]  # holds e

            if g % 2 == 0:
                # m1 for TWO pair-iterations at once (halves its
                # per-instruction overhead; consumed in half-slices).
                xp4 = xb_3d[:, 2 * g : 2 * g + 4, :]
                vj4 = xp4.unsqueeze(-2).broadcast_to((BL, 4, M, M))
                w44 = (
                    w_sb[:, :]
                    .rearrange("p (i j) -> p i j", j=M)
                    .unsqueeze(1)
                    .broadcast_to((BL, 4, M, M))
                )
                nc.vector.tensor_tensor(
                    out=m1[:, :, :, :], in0=w44, in1=vj4, op=mybir.AluOpType.mult
                )
            m1_sl = m1[:, 2 * (g % 2) : 2 * (g % 2) + 2, :, :]

            if VI_ENGINE == "act":
                vi_rep = vi_t[par]
                nc.scalar.copy(out=vi_rep[:, :, :, :], in_=vi)
                vi_in = vi_rep[:, :, :, :]
            else:
                vi_in = vi  # direct broadcast AP (1x mode)
            nc.vector.tensor_tensor(
                out=z_sl, in0=m1_sl, in1=vi_in, op=mybir.AluOpType.mult
            )

            nc.scalar.activation(
                out=e_sl.rearrange("p a i j -> p (a i j)"),
                in_=z_sl.rearrange("p a i j -> p (a i j)"),
                func=mybir.ActivationFunctionType.Exp,
            )

            if g == NPAIR - 1:
                # Early final chain + output DMA for pairs 0..6: the math
                # fills the Vector gap while ACT runs the last pair's exp,
                # and most of the output transfer overlaps the last tree.
                ec = slice(0, (NPAIR - 1) * 2 * M)
                final_chain(ec)
                nc.sync.dma_start(out=out_ext[:, ec], in_=out_sb[:, ec])

            # t = e * v_j overwrites the z slice; rows (s=0) become the
            # numerator stream, rows (s=1) the denominator stream.
            nc.vector.tensor_tensor(
                out=z_sl, in0=e_sl, in1=vj, op=mybir.AluOpType.mult
            )

            # One merged tree over 256 rows = (src 2, k2 2, i 64).
            cur = e2[:, :, :, :, :].rearrange("p s a i j -> p (s a i) j")
            width = M // 2
            while width >= 2:
                nxt = tr2[width][:, :, :]
                nc.vector.tensor_tensor(
                    out=nxt,
                    in0=cur[:, :, 0:width],
                    in1=cur[:, :, width : 2 * width],
                    op=mybir.AluOpType.add,
                )
                cur = nxt
                width //= 2
            cols = slice(g * 2 * M, (g + 1) * 2 * M)
            nc.vector.tensor_tensor(
                out=n_all[:, cols],
                in0=cur[:, 0 : 2 * M, 0],
                in1=cur[:, 0 : 2 * M, 1],
                op=mybir.AluOpType.add,
            )
            nc.vector.tensor_tensor(
                out=d_all[:, cols],
                in0=cur[:, 2 * M : 4 * M, 0],
                in1=cur[:, 2 * M : 4 * M, 1],
                op=mybir.AluOpType.add,
            )

        # Late final chain: only the last pair's 128 columns remain.
        lc = slice((NPAIR - 1) * 2 * M, K * M)
        final_chain(lc)
        nc.sync.dma_start(out=out_ext[:, lc], in_=out_sb[:, lc])

    nc.compile()
    return nc


def _get_nc():
    key = ("nc", VI_ENGINE)
    if key not in _CACHE:
        _CACHE[key] = _build()
    return _CACHE[key]


def kernel(x, vk):
    global LAST_RESULTS
    x = np.ascontiguousarray(np.asarray(x), dtype=np.float32)
    vk = np.ascontiguousarray(np.asarray(vk), dtype=np.float32)
    assert x.shape == (B, M, K) and vk.shape[0] == M

    import ml_dtypes

    bf16 = ml_dtypes.bfloat16
    w = np.ascontiguousarray(
        np.broadcast_to((vk @ vk.T).astype(bf16).reshape(1, M * M), (BL, M * M))
    )
    # k-major bf16 per core: xb[b, k*M + i] = x[b, i, k]
    xs = x.reshape(NCORES, BL, M, K)
    in_maps = []
    for i in range(NCORES):
        xb = np.ascontiguousarray(xs[i].transpose(0, 2, 1)).reshape(BL, K * M)
        in_maps.append({"x": xb.astype(bf16), "w": w})

    nc = _get_nc()
    res = run_bass_kernel_spmd(nc, in_maps, core_ids=list(range(NCORES)), trace=TRACE)
    LAST_RESULTS = res
    outs = []
    for i in range(NCORES):
        o = np.asarray(res.results[i]["out"]).astype(np.float32).reshape(BL, K, M)
        outs.append(o.transpose(0, 2, 1))  # -> [BL, M, K]
    out = np.concatenate(outs, axis=0)
    return np.ascontiguousarray(out).astype(np.float32, copy=False)


# revision 70
# speedup vs baseline: 1.0109x; 1.0102x over previous
"""Trainium2 Bass kernel for nn_AttentionFFM (Vector-engine-bound; ~190-245 us
vs 295-351 us baseline on the same HW epochs).

Reference, per token (b, k) with v = x[b, :, k] (64-vector) and constant
symmetric w = vk @ vk.T:

    e_ij  = exp(v_i * v_j * w_ij)
    out_i = v_i * (sum_j v_j e_ij) / (sum_j e_ij)

The v_i factor is pulled OUT of the softmax-weighted sum, so the naive
kernel's full-size s = v_i*v_j outer product (fp32 strided, 8.4 us/instr)
and q = s*e pass disappear; only j-varying factors touch 64x64 tensors.

Layout (per core; batch-parallel across 8 cores, BL=128 batches each):
partitions = batch b, free = (k2, i, j) for one k-PAIR at a time (k-pair
packing halves instruction count). All big ops are bf16 with stride-1
innermost APs => DVE 2x mode (0.52 ns/elem; no 4x mode exists for
tensor_tensor on this HW). Per pair, with e2 = [src=2, k2, i, j] tile:

    vi_rep   = v_i bcast along j     [ACT Copy: ACT cost is stride-
                                      independent, so the stride-0
                                      broadcast that would force 1x on
                                      DVE rides the half-idle ACT engine]
    m1       = w (*) v_j-bcast       [DVE 2x, broadcast AP is packed]
    e2[:,0]  = m1 (*) vi_rep  (= z)  [DVE 2x]
    e2[:,1]  = exp(e2[:,0])   (= e)  [ACT]
    e2[:,0]  = e2[:,1] (*) v_j (= t) [DVE 2x, overwrites z]
    one merged pairwise-halving bf16 add-tree over e2 rows (s,k2,i)=256
    reduces BOTH tensors' j-dims in 5 double-wide 2x levels, then two
    small adds split the results into n_all / d_all columns.
    (Measured: tensor_reduce is 1x, tensor_tensor_scan 0.5x, GpSimd adds
    4-7 ns/elem, bn_stats caps at 512 free — trees win.)

Final, batched over all 16 k in fp32:
    out = v * N * reciprocal_approx_fast(D)    (5x faster than
                                                vector.reciprocal)

Host-side prep: x is passed k-major ((k,i) columns, bf16) so v-vectors are
contiguous per k; output is returned k-major and transposed back on host.
All tiles are allocated ONCE (tile-pool slot cycling faulted on this HW in
a previous session).
"""

import sys
from contextlib import ExitStack

import numpy as np

if "/opt/trn_rl_repo" not in sys.path:
    sys.path.insert(0, "/opt/trn_rl_repo")

import concourse.bass as bass
import concourse.tile as tile
from concourse import bacc, mybir
from concourse.bass_utils import run_bass_kernel_spmd

_NEFF_CACHE_DIR = "/tmp/bass_neff_cache"


def _install_neff_cache():
    import hashlib
    import shutil

    from concourse import bass_utils as _bu

    if getattr(_bu.compile_bir_kernel, "_is_cached_wrapper", False):
        return

    _orig = _bu.compile_bir_kernel

    _volatile = {"ant_traceback", "filename", "lineno", "kernel_name"}

    def _strip(obj):
        if isinstance(obj, dict):
            return {k: _strip(v) for k, v in obj.items() if k not in _volatile}
        if isinstance(obj, list):
            return [_strip(v) for v in obj]
        return obj

    def _key(bir_json):
        import orjson

        try:
            normalized = orjson.dumps(_strip(orjson.loads(bir_json)))
        except Exception:
            normalized = bir_json
        return hashlib.sha256(normalized).hexdigest()[:32]

    def _cached(bir_json, tmpdir, neff_name="file.neff"):
        import os as _os

        try:
            _os.makedirs(_NEFF_CACHE_DIR, exist_ok=True)
            p = _os.path.join(_NEFF_CACHE_DIR, _key(bir_json) + ".neff")
            dst = _os.path.join(tmpdir, neff_name)
            if _os.path.exists(p):
                shutil.copy(p, dst)
                return dst
            out = _orig(bir_json, tmpdir, neff_name)
            try:
                shutil.copy(out, p)
            except Exception:
                pass
            return out
        except Exception:
            return _orig(bir_json, tmpdir, neff_name)

    _cached._is_cached_wrapper = True
    _bu.compile_bir_kernel = _cached
    try:
        import concourse.bass2jax as _b2j

        if hasattr(_b2j, "compile_bir_kernel"):
            _b2j.compile_bir_kernel = _cached
    except Exception:
        pass


_install_neff_cache()

B, M, K = 1024, 64, 16
NCORES = 8
BL = B // NCORES
NPAIR = K // 2

_CACHE = {}
LAST_RESULTS = None
TRACE = False
# "act": materialize the v_i broadcast with an ACT-engine Copy (stride-
#        independent cost, idle capacity) so the z multiply runs 2x on DVE.
# "vector": read the broadcast AP directly in the z multiply (1x, fallback).
VI_ENGINE = "act"


def _build():
    nc = bacc.Bacc(
        "TRN2",
        target_bir_lowering=False,
        debug=False,
        num_devices=NCORES,
    )
    # x, k-major bf16: xb[b, k*M + i] = x[b, i, k]
    x_in = nc.declare_dram_parameter(
        "x", [BL, K * M], mybir.dt.bfloat16, isOutput=False
    )
    # w arrives pre-broadcast across partitions so the load is a plain
    # contiguous hardware-DGE DMA (the gpsimd software-ring partition
    # broadcast serialized ~4 us into the startup critical path).
    w_in = nc.declare_dram_parameter(
        "w", [BL, M * M], mybir.dt.bfloat16, isOutput=False
    )
    # out, k-major fp32: out[b, k*M + i]
    out_ext = nc.declare_dram_parameter(
        "out", [BL, K * M], mybir.dt.float32, isOutput=True
    )

    with tile.TileContext(nc) as tc, ExitStack() as ctx:
        const = ctx.enter_context(tc.tile_pool(name="const", bufs=1))
        big = ctx.enter_context(tc.tile_pool(name="big", bufs=1))

        xb_sb = const.tile([BL, K * M], mybir.dt.bfloat16)
        nc.sync.dma_start(out=xb_sb[:, :], in_=x_in[:, :])

        w_sb = const.tile([BL, M * M], mybir.dt.bfloat16)
        nc.scalar.dma_start(out=w_sb[:, :], in_=w_in[:, :])

        out_sb = const.tile([BL, K * M], mybir.dt.float32)

        xb_3d = xb_sb[:, :].rearrange("p (k i) -> p k i", i=M)
        w4 = (
            w_sb[:, :]
            .rearrange("p (i j) -> p i j", j=M)
            .unsqueeze(1)
            .broadcast_to((BL, 2, M, M))
        )

        # Ping-pong tiles where producer/consumer engines differ; the m1
        # tile is produced and consumed back-to-back on Vector, so a
        # single buffer costs no overlap.
        vi_t = (
            [
                big.tile(
                    [BL, 2, M, M], mybir.dt.bfloat16, tag=f"vi{p}", name=f"vi{p}"
                )
                for p in range(2)
            ]
            if VI_ENGINE == "act"
            else None
        )
        m1 = big.tile([BL, 2, M, M], mybir.dt.bfloat16, tag="m1", name="m1")
        e2_t = [
            big.tile(
                [BL, 2, 2, M, M], mybir.dt.bfloat16, tag=f"e2{p}", name=f"e2{p}"
            )
            for p in range(2)
        ]
        # One shared merged-tree tile set (all tree levels run on Vector,
        # which serializes them anyway).
        tr2 = {}
        width = M // 2
        while width >= 2:
            tr2[width] = big.tile(
                [BL, 4 * M, width],
                mybir.dt.bfloat16,
                tag=f"tr{width}",
                name=f"tr{width}",
            )
            width //= 2

        d_all = const.tile([BL, K * M], mybir.dt.float32, tag="dall", name="dall")
        n_all = const.tile([BL, K * M], mybir.dt.bfloat16, tag="nall", name="nall")
        rd_all = const.tile([BL, K * M], mybir.dt.float32, tag="rdall", name="rdall")
        p_all = const.tile([BL, K * M], mybir.dt.float32, tag="pall", name="pall")

        for g in range(NPAIR):
            par = g % 2
            xp = xb_3d[:, 2 * g : 2 * g + 2, :]  # [BL, 2, M] contiguous
            vj = xp.unsqueeze(-2).broadcast_to((BL, 2, M, M))  # j innermost, s=1
            vi = xp.unsqueeze(-1).broadcast_to((BL, 2, M, M))  # j innermost, s=0

            e2 = e2_t[par]
            z_sl = e2[:, 0, :, :, :]  # holds z, then t
            e_sl = e2[:, 1, :, :, :]  # holds e

            nc.vector.tensor_tensor(
                out=m1[:, :, :, :], in0=w4, in1=vj, op=mybir.AluOpType.mult
            )

            if VI_ENGINE == "act":
                vi_rep = vi_t[par]
                nc.scalar.copy(out=vi_rep[:, :, :, :], in_=vi)
                vi_in = vi_rep[:, :, :, :]
            else:
                vi_in = vi  # direct broadcast AP (1x mode)
            nc.vector.tensor_tensor(
                out=z_sl, in0=m1[:, :, :, :], in1=vi_in, op=mybir.AluOpType.mult
            )

            nc.scalar.activation(
                out=e_sl.rearrange("p a i j -> p (a i j)"),
                in_=z_sl.rearrange("p a i j -> p (a i j)"),
                func=mybir.ActivationFunctionType.Exp,
            )

            if g == NPAIR - 1:
                # Early final chain for pairs 0..6: queued here so it
                # executes in the Vector gap while ACT runs the last
                # pair's exp (measured ~3.9 us idle otherwise).
                ec = slice(0, (NPAIR - 1) * 2 * M)
                nc.vector.reciprocal_approx_fast(
                    out=rd_all[:, ec], in_=d_all[:, ec]
                )
                nc.vector.tensor_tensor(
                    out=p_all[:, ec],
                    in0=n_all[:, ec],
                    in1=rd_all[:, ec],
                    op=mybir.AluOpType.mult,
                )
                nc.vector.tensor_tensor(
                    out=out_sb[:, ec],
                    in0=p_all[:, ec],
                    in1=xb_sb[:, ec],
                    op=mybir.AluOpType.mult,
                )

            # t = e * v_j overwrites the z slice; rows (s=0) become the
            # numerator stream, rows (s=1) the denominator stream.
            nc.vector.tensor_tensor(
                out=z_sl, in0=e_sl, in1=vj, op=mybir.AluOpType.mult
            )

            # One merged tree over 256 rows = (src 2, k2 2, i 64).
            cur = e2[:, :, :, :, :].rearrange("p s a i j -> p (s a i) j")
            width = M // 2
            while width >= 2:
                nxt = tr2[width][:, :, :]
                nc.vector.tensor_tensor(
                    out=nxt,
                    in0=cur[:, :, 0:width],
                    in1=cur[:, :, width : 2 * width],
                    op=mybir.AluOpType.add,
                )
                cur = nxt
                width //= 2
            cols = slice(g * 2 * M, (g + 1) * 2 * M)
            nc.vector.tensor_tensor(
                out=n_all[:, cols],
                in0=cur[:, 0 : 2 * M, 0],
                in1=cur[:, 0 : 2 * M, 1],
                op=mybir.AluOpType.add,
            )
            nc.vector.tensor_tensor(
                out=d_all[:, cols],
                in0=cur[:, 2 * M : 4 * M, 0],
                in1=cur[:, 2 * M : 4 * M, 1],
                op=mybir.AluOpType.add,
            )

        # Late final chain: only the last pair's 128 columns remain.
        lc = slice((NPAIR - 1) * 2 * M, K * M)
        nc.vector.reciprocal_approx_fast(out=rd_all[:, lc], in_=d_all[:, lc])
        nc.vector.tensor_tensor(
            out=p_all[:, lc],
            in0=n_all[:, lc],
            in1=rd_all[:, lc],
            op=mybir.AluOpType.mult,
        )
        nc.vector.tensor_tensor(
            out=out_sb[:, lc],
            in0=p_all[:, lc],
            in1=xb_sb[:, lc],
            op=mybir.AluOpType.mult,
        )

        nc.sync.dma_start(out=out_ext[:, :], in_=out_sb[:, :])

    nc.compile()
    return nc


def _get_nc():
    key = ("nc", VI_ENGINE)
    if key not in _CACHE:
        _CACHE[key] = _build()
    return _CACHE[key]


def kernel(x, vk):
    global LAST_RESULTS
    x = np.ascontiguousarray(np.asarray(x), dtype=np.float32)
    vk = np.ascontiguousarray(np.asarray(vk), dtype=np.float32)
    assert x.shape == (B, M, K) and vk.shape[0] == M

    import ml_dtypes

    bf16 = ml_dtypes.bfloat16
    w = np.ascontiguousarray(
        np.broadcast_to((vk @ vk.T).astype(bf16).reshape(1, M * M), (BL, M * M))
    )
    # k-major bf16 per core: xb[b, k*M + i] = x[b, i, k]
    xs = x.reshape(NCORES, BL, M, K)
    in_maps = []
    for i in range(NCORES):
        xb = np.ascontiguousarray(xs[i].transpose(0, 2, 1)).reshape(BL, K * M)
        in_maps.append({"x": xb.astype(bf16), "w": w})

    nc = _get_nc()
    res = run_bass_kernel_spmd(nc, in_maps, core_ids=list(range(NCORES)), trace=TRACE)
    LAST_RESULTS = res
    outs = []
    for i in range(NCORES):
        o = np.asarray(res.results[i]["out"]).reshape(BL, K, M)
        outs.append(o.transpose(0, 2, 1))  # -> [BL, M, K]
    out = np.concatenate(outs, axis=0)
    return np.ascontiguousarray(out).astype(np.float32, copy=False)
